# revision 12
# baseline (speedup 1.0000x reference)
"""Causal multi-head attention (B=4, S=2048, D=1024, H=16) on 8 TRN2 cores.

Sharding: core c -> (batch b = c//2, head-group g = c%2, 8 heads each).
Host pre-transposes/splits inputs; device returns per-core partial outputs
y_c = attn_heads(g) @ wo[g-rows]; host sums the two partials per batch.

Precision: q/k projections and QK^T run in single-pass float32r matmuls
(fp32 operands, ~12-bit internal mantissa, full rate at output>=256 rows).
QK^T packs K=128 as [q;q]x[k;k] (computes 2*q.k, folded into softmax scale
1/16), which runs ~2x faster than K=64 on HW. The causal mask, 1/16 scale,
negation, and row-max all fuse into DVE tensor_tensor_reduce passes; exp
runs on ScalarE with scale=-1 and bias=-max. Value path fp16 end-to-end.
Predicted end-to-end rel err ~7e-3 (host-sim with HW-calibrated noise).
"""

import numpy as np

import concourse.bacc as bacc
import concourse.tile as tile
from concourse import mybir
from concourse.bass_utils import run_bass_kernel_spmd

B, S, D = 4, 2048, 1024
H, DK = 16, 64
HL = 8            # heads per core
DL = HL * DK      # 512 local channels
N_CORES = 8
P = 128           # partitions
KT = D // P       # 8 contraction tiles
QT = S // P       # 16 q tiles
MS = 256          # proj m-slab (seq cols per x stage tile)
MT = S // MS      # 8
NT = DL // P      # 4 channel slabs of 128
CHUNK = 1024      # score chunk (2 PSUM banks)
BIG = 1.0e30

f32 = mybir.dt.float32
f32r = mybir.dt.float32r
f16 = mybir.dt.float16
ALU = mybir.AluOpType
AF = mybir.ActivationFunctionType
AX = mybir.AxisListType.X

_cache = {}


def _build():
    nc = bacc.Bacc("TRN2", target_bir_lowering=False)

    def din(name, shape, dt):
        return nc.dram_tensor(name, shape, dt, kind="ExternalInput").ap()

    xq = din("xq", [D, S], f32r)
    xk = din("xk", [D, S], f32r)
    xv = din("xv", [D, S], f16)
    wq = din("wq", [D, DL], f32r)
    wk = din("wk", [D, DL], f32r)
    wv = din("wv", [D, DL], f16)
    wo = din("wo", [DL, D], f16)
    ident = din("ident", [P, P], f32r)
    maskw = din("maskw", [P, 512], f32r)
    y = nc.dram_tensor("y", [S, D], f32, kind="ExternalOutput").ap()

    with tile.TileContext(nc) as tc:
        _body(nc, tc, xq, xk, xv, wq, wk, wv, wo, ident, maskw, y)
    nc.compile()
    return nc


def _body(nc, tc, xq, xk, xv, wq, wk, wv, wo, ident, maskw, y):
    from contextlib import ExitStack
    ctx = ExitStack()
    with ctx:
        # ---------- long-lived tiles ----------
        persist = ctx.enter_context(tc.tile_pool(name="persist", bufs=1))
        # kdup[h]: head h's k channels duplicated on both partition halves
        kdup = [persist.tile([P, S], f32r, tag=f"kd_{h}", name=f"kd_{h}") for h in range(HL)]
        # qsb[n]: heads 2n (p0:64) and 2n+1 (p64:128), channel-major, x1/16
        qsb = [persist.tile([P, S], f32r, tag=f"qs_{n}", name=f"qs_{n}") for n in range(NT)]
        vsb = [persist.tile([P, DL], f16, tag=f"v_{m}", name=f"v_{m}") for m in range(QT)]
        outT = persist.tile([P, NT, S], f16, tag="outT", name="outT")
        ident_sb = persist.tile([P, P], f32r, tag="ident")
        maskw_sb = persist.tile([P, 512], f32r, tag="maskw")
        nc.sync.dma_start(out=ident_sb, in_=ident)
        nc.sync.dma_start(out=maskw_sb, in_=maskw)

        # ---------- phase 1: projections ----------
        with (
            tc.tile_pool(name="wpool", bufs=1) as wpool,
            tc.tile_pool(name="xpool", bufs=2) as xpool,
            tc.tile_pool(name="ppsum", bufs=4, space="PSUM") as ppsum,
        ):
            wv_sb = wpool.tile([P, KT, DL], f16, tag="wv", name="wv")
            nc.sync.dma_start(out=wv_sb, in_=wv.rearrange("(k p) n -> p k n", p=P))
            wq_sb = wpool.tile([P, KT, DL], f32r, tag="wq", name="wq")
            nc.sync.dma_start(out=wq_sb, in_=wq.rearrange("(k p) n -> p k n", p=P))
            wk_sb = wpool.tile([P, KT, DL], f32r, tag="wk", name="wk")
            nc.sync.dma_start(out=wk_sb, in_=wk.rearrange("(k p) n -> p k n", p=P))

            # V projection -> seq-major [S, DL], fp16
            for m in range(QT):
                xvt = xpool.tile([P, KT, P], f16, tag="xv", name="xvt")
                nc.sync.dma_start(
                    out=xvt, in_=xv.rearrange("(k p) s -> p k s", p=P)[:, :, m * P:(m + 1) * P])
                ps = ppsum.tile([P, DL], f32, tag="proj")
                for k in range(KT):
                    nc.tensor.matmul(ps[:], xvt[:, k], wv_sb[:, k],
                                     start=(k == 0), stop=(k == KT - 1))
                nc.scalar.copy(vsb[m][:], ps[:])

            # Q/K projections, channel-major out [DL, S] in m-slabs.
            # x staged in half-KT tiles to fit SBUF while keeping ap=256.
            KH = KT // 2
            for m in range(MT):
                msl = slice(m * MS, (m + 1) * MS)
                xq_h = [xpool.tile([P, KH, MS], f32r, tag="xq", name=f"xq_{half}")
                        for half in range(2)]
                xk_h = [xpool.tile([P, KH, MS], f32r, tag="xk", name=f"xk_{half}")
                        for half in range(2)]
                for half in range(2):
                    ksl = slice(half * KH, (half + 1) * KH)
                    nc.gpsimd.dma_start(
                        out=xq_h[half], in_=xq.rearrange("(k p) s -> p k s", p=P)[:, ksl, msl])
                    nc.gpsimd.dma_start(
                        out=xk_h[half], in_=xk.rearrange("(k p) s -> p k s", p=P)[:, ksl, msl])
                for n in range(NT):
                    csl = slice(n * P, (n + 1) * P)
                    psq = ppsum.tile([P, MS], f32, tag="proj")
                    for k in range(KT):
                        nc.tensor.matmul(psq[:], wq_sb[:, k, csl], xq_h[k // KH][:, k % KH],
                                         start=(k == 0), stop=(k == KT - 1))
                    # pre-scale q by 1/16 so QK^T PSUM holds final scaled scores
                    nc.vector.tensor_scalar_mul(qsb[n][:, msl], psq[:], 0.0625)
                    psk = ppsum.tile([P, MS], f32, tag="proj")
                    for k in range(KT):
                        nc.tensor.matmul(psk[:], wk_sb[:, k, csl], xk_h[k // KH][:, k % KH],
                                         start=(k == 0), stop=(k == KT - 1))
                    h0, h1 = 2 * n, 2 * n + 1
                    nc.scalar.copy(kdup[h0][0:DK, msl], psk[0:DK, :])
                    nc.scalar.copy(kdup[h1][DK:P, msl], psk[DK:P, :])
                    nc.gpsimd.dma_start(out=kdup[h0][DK:P, msl], in_=kdup[h0][0:DK, msl])
                    nc.gpsimd.dma_start(out=kdup[h1][0:DK, msl], in_=kdup[h1][DK:P, msl])

        # ---------- phase 3: attention ----------
        with (
            tc.tile_pool(name="scpool", bufs=3, space="PSUM") as scpool,
            tc.tile_pool(name="pvpool", bufs=2, space="PSUM") as pvpool,
            tc.tile_pool(name="qdpool", bufs=6) as qdpool,
            tc.tile_pool(name="ppool", bufs=6) as ppool,
            tc.tile_pool(name="ptpool", bufs=6) as ptpool,
            tc.tile_pool(name="stat", bufs=6) as stat,
            tc.tile_pool(name="opool", bufs=2) as opool,
            tc.tile_pool(name="ostage", bufs=3) as ostage,
        ):
            iters = [(qt, h) for qt in range(QT) for h in range(HL)]
            LAG = 5
            state = {}

            def get_qt_tiles(qt):
                if qt not in state:
                    klen = (qt + 1) * P
                    nch = 1 if klen <= CHUNK else 2
                    state[qt] = dict(
                        m_t=stat.tile([P, 2 * HL], f32, tag="m1", name="m1t"),
                        z_t=stat.tile([P, 2 * HL], f32, tag="z1", name="z1t"),
                        ostg=ostage.tile([P, DL], f16, tag="ostg", name="ostg"),
                        nch=nch, pt={}, osb={},
                    )
                return state[qt]

            def chunks_of(qt):
                klen = (qt + 1) * P
                return [(0, klen)] if klen <= CHUNK else [(0, CHUNK), (CHUNK, klen)]

            def alpha(qt, h):
                st = get_qt_tiles(qt)
                klen = (qt + 1) * P
                n, hh = h // 2, h % 2
                qdup = qdpool.tile([P, P], f32r, tag="qd", name="qdup")
                qsrc = qsb[n][hh * DK:(hh + 1) * DK, qt * P:(qt + 1) * P]
                nc.gpsimd.dma_start(out=qdup[0:DK, :], in_=qsrc)
                nc.gpsimd.dma_start(out=qdup[DK:P, :], in_=qsrc)
                pc = ppool.tile([P, klen], f16, tag="p", padded_shape=[P, S], name="pc")
                for ci, (c0, c1) in enumerate(chunks_of(qt)):
                    cl = c1 - c0
                    sc = scpool.tile([P, CHUNK], f32, tag="scores", name="sc")
                    if c1 == klen:
                        # final chunk: fold the causal mask into the PSUM
                        # accumulation of the last sub-chunk via ident^T@maskw
                        # (sub-chunks stay 512-aligned: PSUM bank boundaries)
                        last = (cl - 1) // 512 * 512
                        for n0 in range(0, last, 512):
                            nc.tensor.matmul(sc[:, n0:n0 + 512], qdup[:],
                                             kdup[h][:, c0 + n0:c0 + n0 + 512],
                                             start=True, stop=True)
                        nn = cl - last
                        nc.tensor.matmul(sc[:, last:cl], qdup[:],
                                         kdup[h][:, c0 + last:c0 + cl],
                                         start=True, stop=False)
                        nc.tensor.matmul(sc[:, last:cl], ident_sb[:],
                                         maskw_sb[:, 512 - nn:512],
                                         start=False, stop=True)
                    else:
                        for n0 in range(0, cl, 512):
                            nn = min(512, cl - n0)
                            nc.tensor.matmul(sc[:, n0:n0 + nn], qdup[:],
                                             kdup[h][:, c0 + n0:c0 + n0 + nn],
                                             start=True, stop=True)
                    mt = st["m_t"][:, 2 * h + ci:2 * h + ci + 1]
                    zt = st["z_t"][:, 2 * h + ci:2 * h + ci + 1]
                    nc.vector.reduce_max(mt, sc[:, :cl], axis=AX, negate=True)
                    # pc = exp(sc - max), z accum
                    nc.scalar.activation(pc[:, c0:c1], sc[:, :cl], AF.Exp,
                                         bias=mt, scale=1.0, accum_out=zt)
                pt = ptpool.tile([P, QT, P], f16, tag="pt", name="pt")
                st["pt"][h] = pt
                nc.sync.dma_start_transpose(pt[:, 0:klen // P, :], pc[:])

            def beta(qt, h):
                st = get_qt_tiles(qt)
                pt = st["pt"][h]
                for ci, (c0, c1) in enumerate(chunks_of(qt)):
                    nkb = (c1 - c0) // P
                    ops = pvpool.tile([P, DK], f32, tag="pv", name="pvt")
                    for kb in range(nkb):
                        nc.tensor.matmul(
                            ops[:], pt[:, c0 // P + kb, :],
                            vsb[c0 // P + kb][:, h * DK:(h + 1) * DK],
                            start=(kb == 0), stop=(kb == nkb - 1))
                    if st["nch"] == 1:
                        rh = stat.tile([P, 1], f32, tag="rh")
                        nc.vector.reciprocal(rh, st["z_t"][:, 2 * h:2 * h + 1])
                        nc.scalar.activation(
                            st["ostg"][:, h * DK:(h + 1) * DK], ops[:], AF.Copy, scale=rh)
                    else:
                        osb = opool.tile([P, DK], f32, tag=f"o{ci}_{h}", name=f"osb{ci}_{h}")
                        nc.scalar.copy(osb[:], ops[:])
                        st["osb"][(h, ci)] = osb

            def finish_qt(qt):
                st = state[qt]
                ostg = st["ostg"]
                if st["nch"] == 2:
                    m_t, z_t = st["m_t"], st["z_t"]
                    ev = slice(0, 2 * HL, 2)
                    od = slice(1, 2 * HL, 2)
                    m1, m2 = m_t[:, ev], m_t[:, od]   # negated scaled chunk maxes
                    z1, z2 = z_t[:, ev], z_t[:, od]
                    negM = stat.tile([P, HL], f32, tag="negM")
                    nc.vector.tensor_tensor(out=negM, in0=m1, in1=m2, op=ALU.min)
                    d1 = stat.tile([P, HL], f32, tag="d1")
                    d2 = stat.tile([P, HL], f32, tag="d2")
                    nc.vector.tensor_tensor(out=d1, in0=negM, in1=m1, op=ALU.subtract)
                    nc.vector.tensor_tensor(out=d2, in0=negM, in1=m2, op=ALU.subtract)
                    w1 = stat.tile([P, HL], f32, tag="w1")
                    w2 = stat.tile([P, HL], f32, tag="w2")
                    nc.scalar.activation(w1, d1, AF.Exp, scale=1.0)
                    nc.scalar.activation(w2, d2, AF.Exp, scale=1.0)
                    zz = stat.tile([P, HL], f32, tag="zz")
                    zs = stat.tile([P, HL], f32, tag="zs")
                    nc.vector.tensor_tensor(out=zz, in0=w1, in1=z1, op=ALU.mult)
                    nc.vector.tensor_tensor(out=zs, in0=w2, in1=z2, op=ALU.mult)
                    ztot = stat.tile([P, HL], f32, tag="ztot")
                    nc.vector.tensor_tensor(out=ztot, in0=zz, in1=zs, op=ALU.add)
                    r_t = stat.tile([P, HL], f32, tag="r")
                    nc.vector.reciprocal(r_t, ztot)
                    s1 = stat.tile([P, HL], f32, tag="s1")
                    s2 = stat.tile([P, HL], f32, tag="s2")
                    nc.vector.tensor_tensor(out=s1, in0=w1, in1=r_t, op=ALU.mult)
                    nc.vector.tensor_tensor(out=s2, in0=w2, in1=r_t, op=ALU.mult)
                    for h in range(HL):
                        osl = ostg[:, h * DK:(h + 1) * DK]
                        nc.scalar.activation(osl, st["osb"][(h, 0)][:], AF.Copy,
                                             scale=s1[:, h:h + 1])
                        nc.vector.scalar_tensor_tensor(
                            out=osl, in0=st["osb"][(h, 1)][:], scalar=s2[:, h:h + 1],
                            in1=osl, op0=ALU.mult, op1=ALU.add)
                nc.sync.dma_start_transpose(outT[:, :, qt * P:(qt + 1) * P], ostg[:])
                del state[qt]["pt"]

            for i, (qt, h) in enumerate(iters):
                alpha(qt, h)
                if i >= LAG:
                    bqt, bh = iters[i - LAG]
                    beta(bqt, bh)
                    if bh == HL - 1:
                        finish_qt(bqt)
            for j in range(len(iters) - LAG, len(iters)):
                bqt, bh = iters[j]
                beta(bqt, bh)
                if bh == HL - 1:
                    finish_qt(bqt)

        # ---------- phase 4: output projection ----------
        with (
            tc.tile_pool(name="wopool", bufs=1) as wopool,
            tc.tile_pool(name="ypsum", bufs=3, space="PSUM") as ypsum,
            tc.tile_pool(name="ypool", bufs=3) as ypool,
        ):
            wo_sb = wopool.tile([P, NT, D], f16, tag="wo")
            nc.sync.dma_start(out=wo_sb, in_=wo.rearrange("(j p) n -> p j n", p=P))
            for m in range(QT):
                for n in range(2):
                    ps = ypsum.tile([P, 512], f32, tag="yps")
                    for j in range(NT):
                        nc.tensor.matmul(
                            ps[:], outT[:, j, m * P:(m + 1) * P],
                            wo_sb[:, j, n * 512:(n + 1) * 512],
                            start=(j == 0), stop=(j == NT - 1))
                    ysb = ypool.tile([P, 512], f32, tag="y")
                    nc.scalar.copy(ysb[:], ps[:])
                    nc.gpsimd.dma_start(out=y[m * P:(m + 1) * P, n * 512:(n + 1) * 512], in_=ysb[:])


def _host_prep(q, k, v, wq, wk, wv, wo):
    """Build the 8 per-core input maps."""
    ident = np.eye(P, dtype=np.float32)
    maskw = np.zeros((P, 512), np.float32)
    maskw[:, 384:512] = np.triu(np.full((P, P), -BIG, np.float32), k=1)
    in_maps = []
    per_b = {}
    for b in range(B):
        per_b[b] = (
            np.ascontiguousarray(q[b].T.astype(np.float32)),
            np.ascontiguousarray(k[b].T.astype(np.float32)),
            np.ascontiguousarray(v[b].T.astype(np.float32)).astype(np.float16),
        )
    per_g = {}
    for g in range(2):
        cs = slice(g * DL, (g + 1) * DL)
        per_g[g] = (
            np.ascontiguousarray(wq[:, cs].astype(np.float32)),
            np.ascontiguousarray(wk[:, cs].astype(np.float32)),
            np.ascontiguousarray(wv[:, cs]).astype(np.float16),
            np.ascontiguousarray(wo[cs, :]).astype(np.float16),
        )
    for c in range(N_CORES):
        b, g = c // 2, c % 2
        xq_c, xk_c, xv_c = per_b[b]
        wq_c, wk_c, wv_c, wo_c = per_g[g]
        in_maps.append({
            "xq": xq_c, "xk": xk_c, "xv": xv_c,
            "wq": wq_c, "wk": wk_c, "wv": wv_c, "wo": wo_c,
            "ident": ident, "maskw": maskw,
        })
    return in_maps


def kernel(q, k, v, wq, wk, wv, wo):
    if "nc" not in _cache:
        _cache["nc"] = _build()
    nc = _cache["nc"]
    in_maps = _host_prep(np.asarray(q), np.asarray(k), np.asarray(v),
                         np.asarray(wq), np.asarray(wk), np.asarray(wv),
                         np.asarray(wo))
    res = run_bass_kernel_spmd(nc, in_maps, list(range(N_CORES)))
    out = np.empty((B, S, D), np.float32)
    for b in range(B):
        out[b] = res.results[2 * b]["y"] + res.results[2 * b + 1]["y"]
    return out


if __name__ == "__main__":
    d = np.load("/root/problem/inputs_cache.npz")
    out = kernel(d["q"], d["k"], d["v"], d["wq"], d["wk"], d["wv"], d["wo"])
    ref = d["ref"]
    rel = np.linalg.norm(out - ref) / np.linalg.norm(ref)
    print(f"Relative error: {rel:.4e}")
